# revision 10
# baseline (speedup 1.0000x reference)
"""GPT-NeoX attention layer on 8 Trainium2 NeuronCores.

Tensor-parallel over heads: each core owns 4 of the 32 heads. Per core:
  1. qT = rope(Wq_c.T @ X^T + bq)   [512, 4096]  (feature-major, fp32r)
  2. kT = rope(Wk_c.T @ X^T + bk)   [512, 4096]
  3. V  = X @ Wv_c + bv             [4096, 512]  (token-major)
  4. causal attention per (batch, head): S^T = kT.T q, P^T = exp(S^T + mask),
     O^T = V.T P^T, denominators via ones-matmul + reciprocal.
  5. out_partial = attn @ Wo_c rows  [4096, 4096]
Host sums the 8 partial outputs (row-parallel all-reduce) and adds b_o.

All matmuls run in float32r (TF32-like, 1 cycle/row at N>=256).
RoPE is fused into the Q/K eviction: rotate-half built with two SBUF->SBUF
DMAs (partition swap), sign folded into the host [-sin;+sin] table.
"""
import sys
sys.path.insert(0, '/opt/trn_rl_repo')
import os
import numpy as np

B, S, HID, H, D = 2, 2048, 4096, 32, 128
NCORES = 8
HPC = H // NCORES            # heads per core = 4
FPC = HPC * D                # per-core q/k/v feature count = 512
BT = B * S                   # 4096 tokens
KT = HID // 128              # 32 contraction tiles
ROPE_BASE = 10000.0
HALF = D // 2
NEG = -1e30

_CACHE = {}


def _build_nc():
    import concourse.bacc as bacc
    import concourse.tile as tile
    from concourse import mybir

    F32R = mybir.dt.float32r
    F32 = mybir.dt.float32
    AF = mybir.ActivationFunctionType

    nc = bacc.Bacc('TRN2', target_bir_lowering=False, debug=False,
                   num_devices=NCORES)

    XT_d = nc.dram_tensor("xt", [HID, BT], F32R, kind="ExternalInput")
    Wq_d = nc.dram_tensor("wq", [HID, FPC], F32R, kind="ExternalInput")
    Wk_d = nc.dram_tensor("wk", [HID, FPC], F32R, kind="ExternalInput")
    Wv_d = nc.dram_tensor("wv", [HID, FPC], F32R, kind="ExternalInput")
    Wo_d = nc.dram_tensor("wo", [FPC, HID], F32R, kind="ExternalInput")
    bq_d = nc.dram_tensor("bq", [128, HPC], F32, kind="ExternalInput")
    bk_d = nc.dram_tensor("bk", [128, HPC], F32, kind="ExternalInput")
    bvb_d = nc.dram_tensor("bvb", [128, FPC], F32, kind="ExternalInput")
    ccq_d = nc.dram_tensor("ccq", [128, S], F32, kind="ExternalInput")
    snq_d = nc.dram_tensor("snq", [128, S], F32, kind="ExternalInput")
    cck_d = nc.dram_tensor("cck", [128, S], F32, kind="ExternalInput")
    snk_d = nc.dram_tensor("snk", [128, S], F32, kind="ExternalInput")
    mask_d = nc.dram_tensor("mask", [128, 4 * 512], F32, kind="ExternalInput")
    ones_col_d = nc.dram_tensor("ones_col", [128, 1], F32R, kind="ExternalInput")
    ones_row_d = nc.dram_tensor("ones_row", [64, 128], F32R, kind="ExternalInput")
    out_d = nc.dram_tensor("out", [BT, HID], F32, kind="ExternalOutput")

    qT_d = nc.dram_tensor("qT_i", [FPC, BT], F32R, kind="Internal")
    kT_d = nc.dram_tensor("kT_i", [FPC, BT], F32R, kind="Internal")
    V_d = nc.dram_tensor("v_i", [BT, FPC], F32R, kind="Internal")
    attnT_d = nc.dram_tensor("attnT_i", [FPC, BT], F32R, kind="Internal")

    def chunked_load(w_s, dram_r, n_ch, dim):
        step = w_s.shape[dim] // n_ch
        for ch in range(n_ch):
            if dim == 1:
                nc.sync.dma_start(w_s[:, ch * step:(ch + 1) * step, :],
                                  dram_r[:, ch * step:(ch + 1) * step, :])
            else:
                nc.sync.dma_start(w_s[:, :, ch * step:(ch + 1) * step],
                                  dram_r[:, :, ch * step:(ch + 1) * step])

    with tile.TileContext(nc) as tc:
        with (
            nc.allow_low_precision(reason="fp32r matmul pipeline"),
            tc.tile_pool(name="konst", bufs=1) as kp,
        ):
            ones_col = kp.tile([128, 1], F32R)
            ones_row = kp.tile([64, 128], F32R)
            nc.sync.dma_start(ones_col[:], ones_col_d[:])
            nc.sync.dma_start(ones_row[:], ones_row_d[:])

            # ---------------- passes 1 & 2: qT / kT with fused RoPE ---------
            for pi, (W_d, b_d, o_d, cc_d, sn_d) in enumerate(
                    [(Wq_d, bq_d, qT_d, ccq_d, snq_d),
                     (Wk_d, bk_d, kT_d, cck_d, snk_d)]):
                with (
                    tc.tile_pool(name=f"w{pi}", bufs=1) as wp,
                    tc.tile_pool(name=f"x{pi}", bufs=3) as xp,
                    tc.tile_pool(name=f"c{pi}", bufs=1) as cp,
                    tc.tile_pool(name=f"t{pi}", bufs=3) as tp,
                    tc.tile_pool(name=f"y{pi}", bufs=3) as yp,
                    tc.tile_pool(name=f"m{pi}", bufs=3) as mp,
                    tc.tile_pool(name=f"ev{pi}", bufs=3) as ep,
                    tc.tile_pool(name=f"ps{pi}", bufs=3, space="PSUM") as pp,
                ):
                    w_s = wp.tile([128, KT, FPC], F32R, tag="w")
                    chunked_load(w_s, W_d[:].rearrange("(kt p) f -> p kt f",
                                                       p=128), 4, 1)
                    b_s = cp.tile([128, HPC], F32, tag="b")
                    nc.sync.dma_start(b_s[:], b_d[:])
                    cc_s = cp.tile([128, S], F32, tag="cc")
                    nc.sync.dma_start(cc_s[:], cc_d[:])
                    sn_s = cp.tile([128, S], F32, tag="sn")
                    nc.sync.dma_start(sn_s[:], sn_d[:])
                    for tb in range(BT // 512):
                        xh = []
                        for hf in range(2):
                            x_s = xp.tile([128, KT // 2, 512], F32R, tag="x")
                            nc.sync.dma_start(
                                x_s[:],
                                XT_d[hf * 2048:(hf + 1) * 2048,
                                     tb * 512:(tb + 1) * 512].rearrange(
                                    "(kt p) n -> p kt n", p=128))
                            xh.append(x_s)
                        pos = (tb % (S // 512)) * 512
                        for f in range(HPC):
                            ps = pp.tile([128, 512], F32, tag="ps")
                            for kt in range(KT):
                                nc.tensor.matmul(
                                    ps[:],
                                    w_s[:, kt, f * 128:(f + 1) * 128],
                                    xh[kt // 16][:, kt % 16, :],
                                    start=(kt == 0), stop=(kt == KT - 1))
                            t = tp.tile([128, 512], F32, tag="t")
                            nc.scalar.activation(
                                t[:], ps[:], AF.Identity,
                                bias=b_s[:, f:f + 1], scale=1.0)
                            y = yp.tile([128, 512], F32, tag="y")
                            nc.sync.dma_start(y[0:64, :], t[64:128, :])
                            nc.sync.dma_start(y[64:128, :], t[0:64, :])
                            ev = ep.tile([128, 512], F32R, tag="ev")
                            nc.vector.tensor_mul(
                                ev[:], t[:], cc_s[:, pos:pos + 512])
                            m2 = mp.tile([128, 512], F32, tag="m2")
                            nc.vector.tensor_mul(
                                m2[:], y[:], sn_s[:, pos:pos + 512])
                            nc.vector.tensor_add(ev[:], ev[:], m2[:])
                            nc.sync.dma_start(
                                o_d[f * 128:(f + 1) * 128,
                                    tb * 512:(tb + 1) * 512], ev[:])

            # ---------------- pass 3: V projection (token-major) ------------
            with (
                tc.tile_pool(name="wv", bufs=1) as wp,
                tc.tile_pool(name="xv", bufs=3) as xp,
                tc.tile_pool(name="bv", bufs=1) as bp,
                tc.tile_pool(name="evv", bufs=3) as ep,
                tc.tile_pool(name="psv", bufs=3, space="PSUM") as pp,
            ):
                w_s = wp.tile([128, KT, FPC], F32R, tag="wv")
                chunked_load(w_s, Wv_d[:].rearrange("(kt p) f -> p kt f",
                                                    p=128), 4, 1)
                bvb_s = bp.tile([128, FPC], F32, tag="bvb")
                nc.sync.dma_start(bvb_s[:], bvb_d[:])
                for tb in range(BT // 512):
                    xh = []
                    for hf in range(2):
                        x_s = xp.tile([128, KT // 2, 512], F32R, tag="xv")
                        nc.sync.dma_start(
                            x_s[:],
                            XT_d[hf * 2048:(hf + 1) * 2048,
                                 tb * 512:(tb + 1) * 512].rearrange(
                                "(kt p) n -> p kt n", p=128))
                        xh.append(x_s)
                    for t in range(4):
                        ps = pp.tile([128, 512], F32, tag="psv")
                        for kt in range(KT):
                            nc.tensor.matmul(
                                ps[:],
                                xh[kt // 16][:, kt % 16, t * 128:(t + 1) * 128],
                                w_s[:, kt, :],
                                start=(kt == 0), stop=(kt == KT - 1))
                        ev = ep.tile([128, 512], F32R, tag="evv")
                        nc.vector.tensor_add(ev[:], ps[:], bvb_s[:])
                        nc.sync.dma_start(
                            V_d[tb * 512 + t * 128: tb * 512 + (t + 1) * 128, :],
                            ev[:])

            # ------- phases 4+5 share one pool for the preloaded W_o --------
            with tc.tile_pool(name="wo", bufs=1) as wop:
                wo_s = wop.tile([128, FPC // 128, HID], F32R, tag="wo")
                chunked_load(wo_s, Wo_d[:].rearrange("(a p) o -> p a o",
                                                     p=128), 4, 2)

                # ------------ phase 4: causal attention per (b, h) ----------
                with (
                    tc.tile_pool(name="tabs", bufs=1) as tabp,
                    tc.tile_pool(name="qk", bufs=2) as qkp,
                    tc.tile_pool(name="vs", bufs=2) as vsp,
                    tc.tile_pool(name="pt", bufs=4) as ptp,
                    tc.tile_pool(name="bsm", bufs=2) as bsp,
                    tc.tile_pool(name="ot", bufs=3) as otp,
                    tc.tile_pool(name="sps", bufs=3, space="PSUM") as sps,
                    tc.tile_pool(name="ops", bufs=2, space="PSUM") as ops,
                    tc.tile_pool(name="rsps", bufs=2, space="PSUM") as rsps,
                    tc.tile_pool(name="bcps", bufs=1, space="PSUM") as bcps,
                ):
                    mask_s = tabp.tile([128, 4 * 512], F32, tag="mask")
                    nc.sync.dma_start(mask_s[:], mask_d[:])

                    for b in range(B):
                        for h in range(HPC):
                            qs = qkp.tile([128, S], F32R, tag="qs")
                            nc.sync.dma_start(
                                qs[:], qT_d[h * 128:(h + 1) * 128,
                                            b * S:(b + 1) * S])
                            ks = qkp.tile([128, S], F32R, tag="ks")
                            nc.sync.dma_start(
                                ks[:], kT_d[h * 128:(h + 1) * 128,
                                            b * S:(b + 1) * S])
                            vs = vsp.tile([128, S // 128, 128], F32R, tag="vs")
                            nc.sync.dma_start(
                                vs[:],
                                V_d[b * S:(b + 1) * S,
                                    h * 128:(h + 1) * 128].rearrange(
                                        "(t p) f -> p t f", p=128))

                            for qb in range(S // 512):
                                nkt = 4 * (qb + 1)
                                o_ps = ops.tile([128, 512], F32, tag="o")
                                rs_ps = rsps.tile([1, 512], F32, tag="rs")
                                s_tiles = {}

                                def emit_s(kt, qb=qb, nkt=nkt,
                                           s_tiles=s_tiles, qs=qs, ks=ks):
                                    sp = sps.tile([128, 512], F32, tag="s")
                                    nc.tensor.matmul(
                                        sp[:],
                                        ks[:, kt * 128:(kt + 1) * 128],
                                        qs[:, qb * 512:(qb + 1) * 512],
                                        start=True, stop=True)
                                    if kt >= nkt - 4:
                                        sub = kt - 4 * qb
                                        nc.vector.tensor_add(
                                            sp[:], sp[:],
                                            mask_s[:,
                                                   sub * 512:(sub + 1) * 512])
                                    s_tiles[kt] = sp

                                emit_s(0)
                                if nkt > 1:
                                    emit_s(1)
                                for kt in range(nkt):
                                    if kt + 2 < nkt:
                                        emit_s(kt + 2)
                                    pt = ptp.tile([128, 512], F32R, tag="pt")
                                    nc.scalar.activation(
                                        pt[:], s_tiles.pop(kt)[:], AF.Exp)
                                    nc.tensor.matmul(
                                        o_ps[:], vs[:, kt, :], pt[:],
                                        start=(kt == 0), stop=(kt == nkt - 1))
                                    nc.tensor.matmul(
                                        rs_ps[:], ones_col[:], pt[:],
                                        start=(kt == 0), stop=(kt == nkt - 1))
                                rcp = bsp.tile([1, 512], F32R, tag="rcp")
                                nc.vector.reciprocal(rcp[:], rs_ps[:])
                                bc_ps = bcps.tile([128, 512], F32, tag="bc")
                                nc.tensor.matmul(
                                    bc_ps[:], ones_row[0:1, :], rcp[:],
                                    start=True, stop=True)
                                bc_s = bsp.tile([128, 512], F32, tag="bcs")
                                nc.vector.tensor_copy(bc_s[:], bc_ps[:])
                                o_t = otp.tile([128, 512], F32R, tag="ot")
                                nc.vector.tensor_mul(o_t[:], o_ps[:], bc_s[:])
                                nc.sync.dma_start(
                                    attnT_d[h * 128:(h + 1) * 128,
                                            b * S + qb * 512:
                                            b * S + (qb + 1) * 512],
                                    o_t[:])

                # ------------ phase 5: output projection --------------------
                with (
                    tc.tile_pool(name="at", bufs=3) as atp,
                    tc.tile_pool(name="cev", bufs=3) as ep,
                    tc.tile_pool(name="cps", bufs=4, space="PSUM") as pp,
                ):
                    for tt in range(BT // 128):
                        a_t = atp.tile([128, FPC // 128, 128], F32R, tag="at")
                        nc.sync.dma_start(
                            a_t[:],
                            attnT_d[:, tt * 128:(tt + 1) * 128].rearrange(
                                "(a p) t -> p a t", p=128))
                        ev = ep.tile([128, HID], F32, tag="cev")
                        for ob in range(HID // 512):
                            ps = pp.tile([128, 512], F32, tag="cps")
                            for a in range(FPC // 128):
                                nc.tensor.matmul(
                                    ps[:], a_t[:, a, :],
                                    wo_s[:, a, ob * 512:(ob + 1) * 512],
                                    start=(a == 0), stop=(a == FPC // 128 - 1))
                            sl = ev[:, ob * 512:(ob + 1) * 512]
                            if ob % 2 == 0:
                                nc.vector.tensor_copy(sl, ps[:])
                            else:
                                nc.scalar.activation(sl, ps[:], AF.Copy)
                        nc.sync.dma_start(
                            out_d[tt * 128:(tt + 1) * 128, :], ev[:])

    nc.compile()
    return nc


def kernel(positions, hidden_states, W_qkv, b_qkv, W_o, b_o):
    from concourse.bass_utils import run_bass_kernel_spmd

    if 'nc' not in _CACHE:
        _CACHE['nc'] = _build_nc()
    nc = _CACHE['nc']

    pos = np.asarray(positions)
    X = np.asarray(hidden_states, dtype=np.float32).reshape(BT, HID)
    W_qkv = np.asarray(W_qkv, dtype=np.float32)
    b_qkv = np.asarray(b_qkv, dtype=np.float32)
    W_o = np.asarray(W_o, dtype=np.float32)
    b_o = np.asarray(b_o, dtype=np.float32)

    XT = np.ascontiguousarray(X.T)                       # [HID, BT]

    inv_freq = (ROPE_BASE ** (-np.arange(HALF, dtype=np.float64) / HALF))
    freqs = pos.astype(np.float64)[:, None] * inv_freq[None, :]   # [S, 64]
    cos = np.cos(freqs).T.astype(np.float32)             # [64, S]
    sin = np.sin(freqs).T.astype(np.float32)
    scale = np.float32(D ** -0.5)
    ccq = np.ascontiguousarray(np.concatenate([cos, cos], 0) * scale)
    snq = np.ascontiguousarray(np.concatenate([-sin, sin], 0) * scale)
    cck = np.ascontiguousarray(np.concatenate([cos, cos], 0))
    snk = np.ascontiguousarray(np.concatenate([-sin, sin], 0))

    k_idx = np.arange(128)[:, None]
    q_idx = np.arange(512)[None, :]
    mask = np.concatenate(
        [np.where(s_ * 128 + k_idx <= q_idx, 0.0, NEG).astype(np.float32)
         for s_ in range(4)], axis=1)                    # [128, 2048]

    ones_col = np.ones((128, 1), np.float32)
    ones_row = np.ones((64, 128), np.float32)

    in_maps = []
    for c in range(NCORES):
        sl = slice(512 * c, 512 * (c + 1))
        in_maps.append({
            "xt": XT,
            "wq": np.ascontiguousarray(W_qkv[:, sl]),
            "wk": np.ascontiguousarray(W_qkv[:, 4096:][:, sl]),
            "wv": np.ascontiguousarray(W_qkv[:, 8192:][:, sl]),
            "wo": np.ascontiguousarray(W_o[sl, :]),
            "bq": np.ascontiguousarray(b_qkv[sl].reshape(HPC, 128).T),
            "bk": np.ascontiguousarray(b_qkv[4096:][sl].reshape(HPC, 128).T),
            "bvb": np.ascontiguousarray(
                np.tile(b_qkv[8192:][sl][None, :], (128, 1))),
            "ccq": ccq, "snq": snq, "cck": cck, "snk": snk,
            "mask": mask,
            "ones_col": ones_col, "ones_row": ones_row,
        })

    res = run_bass_kernel_spmd(nc, in_maps, core_ids=list(range(NCORES)))
    kernel._last_results = res

    total = res.results[0]["out"].astype(np.float32).copy()
    for c in range(1, NCORES):
        total += res.results[c]["out"]
    total += b_o[None, :]
    return total.reshape(B, S, HID)
